# revision 22
# baseline (speedup 1.0000x reference)
"""AutoMTLSuperNet (moe_routing) Trainium2 kernel.

Strategy: batch data-parallel over 8 NeuronCores (2048 samples each, params
replicated). On-chip layout is output-channel-major ([oc, batch]); all
matmuls run in bf16 with f32 PSUM accumulation; batch in chunks of 512 cols.

v2 optimizations over the first working version:
- Candidate-branch psum outputs allocated as [128,1024] PAIR tiles spanning
  two psum banks, evicted with single wide ACT/DVE instructions (amortizes
  the fixed access latency of ScalarE/VectorE).
- relu branch scale-folded into the eviction (softmax mix weight > 0).
- gelu/tanh branches: ACT eviction + one fused scalar_tensor_tensor
  accumulate on DVE (bf16 SBUF 2x mode) instead of mult+add.
- Expert-mix gate rows broadcast across partitions with DMA (0-stride free
  dim) instead of PE selector matmuls + ACT psum copies; softmax reciprocal
  folded into the gate rows before broadcast.
- Gate/s/sq matmuls merged into two stationaries; zero-padding ktile
  dropped (gates read the hybrid tile's sparse rows).
- Domain one-hot computed host-side (kills per-tile is_equal masks).
"""

import numpy as np
import ml_dtypes

import concourse.bass as bass
import concourse.bacc as bacc
import concourse.mybir as mybir
import concourse.tile as tile
from concourse.bass_utils import run_bass_kernel_spmd

# ---- problem dims (hardcoded per contract) ----
B, F, E, D = 16384, 26, 16, 13
NE, ND, NC = 4, 3, 3
GIN = E * (F + 1) + D            # 445
H, OUT = 256, 128
N_CORES = 8
B_LOC = B // N_CORES             # 2048
NBC = 512                        # batch columns per chunk
NCHUNK = B_LOC // NBC            # 4
KSP = F * E                      # 416 flattened sparse dim
BF16 = mybir.dt.bfloat16
F32 = mybir.dt.float32

AF = mybir.ActivationFunctionType
ALU = mybir.AluOpType


def _bf16(x):
    return np.asarray(x, dtype=ml_dtypes.bfloat16)


def _softmax_np(a):
    a = np.asarray(a, dtype=np.float64)
    m = a.max(axis=-1, keepdims=True)
    e = np.exp(a - m)
    return (e / e.sum(axis=-1, keepdims=True)).astype(np.float32)


def prep_shared(inputs):
    """Host prep of all parameter tensors (input-layout + parameter-only math)."""
    f32 = np.float32
    gate_w = 1.0 / (1.0 + np.exp(-inputs['feat_alpha'].astype(np.float64)))  # [NE,F]
    gate_w = gate_w.astype(f32)

    W_l0b0 = inputs['W_l0b0'].astype(f32)   # [NE,NC,GIN,H]
    W_l0b1 = inputs['W_l0b1'].astype(f32)   # [NE,NC,H,OUT]
    W_l1b0 = inputs['W_l1b0'].astype(f32)   # [NE,NC,OUT,H]
    W_l1b1 = inputs['W_l1b1'].astype(f32)   # [NE,NC,H,OUT]

    # candidate softmax weights per mixed-op layer: [4][NE,NC]
    wmix_l = [_softmax_np(inputs[k]) for k in ('a_l0b0', 'a_l0b1', 'a_l1b0', 'a_l1b1')]

    # ---- Wl0: lhsT ktiles [4,128, 3072]; col = n*768 + c*256 + h ----
    Wl0 = np.zeros((4, 128, NE * NC * H), dtype=f32)
    Wsp = np.zeros((KSP, NE, NC, H), dtype=f32)
    for n in range(NE):
        gvec = np.repeat(gate_w[n], E)                      # [416]
        Wsp[:, n] = W_l0b0[n, :, :KSP, :].transpose(1, 0, 2) * gvec[:, None, None]
    Wsp = Wsp.reshape(KSP, NE * NC * H)
    for kt in range(3):
        Wl0[kt, :, :] = Wsp[kt * 128:(kt + 1) * 128]
    # kt3 (hyb) layout: [0:32]=sparse rows 384..415, [32:45]=dense,
    #                   [45:64]=0, [64:128]=fm rows (64 + n*16 + e)
    Wl0[3, 0:32, :] = Wsp[384:416]
    for d in range(D):
        Wl0[3, 32 + d, :] = W_l0b0[:, :, KSP + E + d, :].reshape(-1)
    for n in range(NE):
        for e in range(E):
            Wl0[3, 64 + n * 16 + e, n * 768:(n + 1) * 768] = \
                W_l0b0[n, :, KSP + e, :].reshape(768)

    # ---- GsWg [4,128,108]: cols 0:64 s-rows (n*16+e); 64:80 g0 (n*4+e);
    #      80:96 zero pad (32-partition alignment); 96:108 g1 (d*4+e).
    #      kt3 rows 0:32 only. ----
    Wg0, Wg1 = inputs['Wg0'].astype(f32), inputs['Wg1'].astype(f32)
    GsWg = np.zeros((4, 128, 108), dtype=f32)
    Gq = np.zeros((4, 128, 64), dtype=f32)
    for fe in range(KSP):
        kt, i = divmod(fe, 128)
        f_, e_ = divmod(fe, E)
        for n in range(NE):
            g = gate_w[n, f_]
            GsWg[kt, i, n * 16 + e_] = g
            Gq[kt, i, n * 16 + e_] = 0.5 * g * g
        for n in range(NE):
            for e in range(NE):
                GsWg[kt, i, 64 + n * 4 + e] = Wg0[n, fe, e]
        for d in range(ND):
            for e in range(NE):
                GsWg[kt, i, 96 + d * 4 + e] = Wg1[d, fe, e]
    gbias0 = np.zeros((16, 1), dtype=f32)
    for n in range(NE):
        for e in range(NE):
            gbias0[n * 4 + e, 0] = inputs['bg0'][n, e] + inputs['beta0'][n, e]
    gbias1 = np.zeros((12, 1), dtype=f32)
    for d in range(ND):
        for e in range(NE):
            gbias1[d * 4 + e, 0] = inputs['bg1'][d, e] + inputs['beta1'][d, e]
    # sel16 [16,4]: row n*4+e -> col n
    sel16 = np.zeros((16, 4), dtype=f32)
    for n in range(NE):
        for e in range(NE):
            sel16[n * 4 + e, n] = 1.0

    # ---- later layer weights ----
    Wb1 = np.zeros((NE, H, NC * OUT), dtype=f32)       # lhsT col = c*128+o
    for n in range(NE):
        Wb1[n] = W_l0b1[n].transpose(1, 0, 2).reshape(H, NC * OUT)
    W10 = np.zeros((NE, OUT, NC * H), dtype=f32)       # col = c*256+h
    for n in range(NE):
        W10[n] = W_l1b0[n].transpose(1, 0, 2).reshape(OUT, NC * H)
    W11 = np.zeros((NE, H, NC * OUT), dtype=f32)
    for n in range(NE):
        W11[n] = W_l1b1[n].transpose(1, 0, 2).reshape(H, NC * OUT)

    wmix = np.zeros((128, 48), dtype=f32)
    for li, wl in enumerate(wmix_l):
        for n in range(NE):
            for c in range(NC):
                wmix[:, li * 12 + n * 3 + c] = wl[n, c]

    ident = np.eye(128, dtype=f32)
    # domain-gate reducer: col 0 = total over 12 rows; col 32+e = sum over d
    s12 = np.zeros((12, 36), dtype=f32)
    for d in range(ND):
        for e in range(NE):
            s12[d * 4 + e, 0] = 1.0
            s12[d * 4 + e, 32 + e] = 1.0

    # pack b1/l1 weights along free axis: fewer, wider DMAs
    Wb1P = np.ascontiguousarray(
        Wb1.reshape(NE, 2, 128, 384).transpose(2, 0, 1, 3).reshape(128, NE * 2 * 384))
    W10P = np.ascontiguousarray(W10.transpose(1, 0, 2).reshape(128, NE * 768))
    W11P = np.ascontiguousarray(
        W11.reshape(NE, 2, 128, 384).transpose(2, 0, 1, 3).reshape(128, NE * 2 * 384))
    shared = {
        'Wl0': _bf16(Wl0), 'GsWg': _bf16(GsWg), 'Gq': _bf16(Gq),
        'sel16': _bf16(sel16), 'Wb1': _bf16(Wb1P), 'W10': _bf16(W10P),
        'W11': _bf16(W11P), 'gbias0': gbias0, 'gbias1': gbias1,
        'wmix': wmix, 'ident': _bf16(ident), 's12': _bf16(s12),
    }
    return shared


def prep_core(inputs, r):
    """Per-core input shards (layout only)."""
    lo, hi = r * B_LOC, (r + 1) * B_LOC
    xs = inputs['sparse_embs'][lo:hi].reshape(B_LOC, KSP)      # [2048,416] f32
    xT = xs.T.astype(np.float32).reshape(KSP, NCHUNK, NBC).transpose(1, 0, 2)
    # xTk[ch][p, kt*512+b] = xT[ch][kt*128+p, b]  (kt 0..2)
    xTk = np.ascontiguousarray(
        xT[:, 0:384].reshape(NCHUNK, 3, 128, NBC).transpose(0, 2, 1, 3)
        .reshape(NCHUNK, 128, 3 * NBC))
    xTs = np.ascontiguousarray(xT[:, 384:416])                 # [4,32,512]
    dxT = inputs['dense_features'][lo:hi].astype(np.float32).T  # [13,2048]
    dxT = _bf16(np.ascontiguousarray(
        dxT.reshape(D, NCHUNK, NBC).transpose(1, 0, 2)))        # [4,13,512]
    dom = inputs['domain_ids'][lo:hi].astype(np.int64)
    oh3 = np.zeros((B_LOC, ND), dtype=np.float32)
    oh3[np.arange(B_LOC), dom] = 1.0
    # oh12[ch][4d+e, b] = (domain[ch*512+b] == d)
    ohT = oh3.T.reshape(ND, NCHUNK, NBC).transpose(1, 0, 2)    # [4,3,512]
    oh12 = np.repeat(ohT, NE, axis=1)                          # [4,12,512]
    return {'xTk': _bf16(xTk), 'xTs': _bf16(xTs), 'dxT': dxT,
            'oh12': _bf16(np.ascontiguousarray(oh12))}


def build_program():
    """Optimized program; assumes all b_l* expert biases are zero."""
    nc = bacc.Bacc(trn_type="TRN2", target_bir_lowering=False, debug=False)

    # ---- DRAM I/O ----
    t_xTk = nc.dram_tensor('xTk', [NCHUNK, 128, 3 * NBC], BF16, kind="ExternalInput").ap()
    t_xTs = nc.dram_tensor('xTs', [NCHUNK, 32, NBC], BF16, kind="ExternalInput").ap()
    t_dxT = nc.dram_tensor('dxT', [NCHUNK, D, NBC], BF16, kind="ExternalInput").ap()
    t_oh12 = nc.dram_tensor('oh12', [NCHUNK, 12, NBC], BF16, kind="ExternalInput").ap()
    t_Wl0 = nc.dram_tensor('Wl0', [4, 128, 3072], BF16, kind="ExternalInput").ap()
    t_GsWg = nc.dram_tensor('GsWg', [4, 128, 108], BF16, kind="ExternalInput").ap()
    t_Gq = nc.dram_tensor('Gq', [4, 128, 64], BF16, kind="ExternalInput").ap()
    t_sel16 = nc.dram_tensor('sel16', [16, 4], BF16, kind="ExternalInput").ap()
    t_Wb1 = nc.dram_tensor('Wb1', [128, NE * 2 * 384], BF16, kind="ExternalInput").ap()
    t_W10 = nc.dram_tensor('W10', [128, NE * 768], BF16, kind="ExternalInput").ap()
    t_W11 = nc.dram_tensor('W11', [128, NE * 2 * 384], BF16, kind="ExternalInput").ap()
    t_gb0 = nc.dram_tensor('gbias0', [16, 1], F32, kind="ExternalInput").ap()
    t_gb1 = nc.dram_tensor('gbias1', [12, 1], F32, kind="ExternalInput").ap()
    t_wmix = nc.dram_tensor('wmix', [128, 48], F32, kind="ExternalInput").ap()
    t_ident = nc.dram_tensor('ident', [128, 128], BF16, kind="ExternalInput").ap()
    t_s12 = nc.dram_tensor('s12', [12, 36], BF16, kind="ExternalInput").ap()
    t_out = nc.dram_tensor('out', [NCHUNK, 128, 4 * OUT], F32, kind="ExternalOutput").ap()

    with tile.TileContext(nc) as tc:
        with (
            tc.tile_pool(name="wpool", bufs=1) as wpool,
            tc.tile_pool(name="xpool", bufs=3) as xpool,
            tc.tile_pool(name="qpool", bufs=2) as qpool,
            tc.tile_pool(name="gpool", bufs=4) as gpool,
            tc.tile_pool(name="spool", bufs=2) as spool,
            tc.tile_pool(name="apool", bufs=2) as apool,
            tc.tile_pool(name="hpool", bufs=2) as hpool,
            tc.tile_pool(name="bcpool", bufs=2) as bcpool,
            tc.tile_pool(name="bigpool", bufs=1) as bigpool,
            tc.tile_pool(name="opool", bufs=2) as opool,
            tc.tile_pool(name="ps_mm", bufs=3, space="PSUM") as ps_mm,
            tc.tile_pool(name="ps_smlt", bufs=2, space="PSUM") as ps_smlt,
        ):
            # ---- prologue: resident weights/constants ----
            def wtile(src_ap, shape, dtype=BF16, tag=None):
                t = wpool.tile(shape, dtype, tag=tag, name=tag)
                nc.sync.dma_start(t[:], src_ap)
                return t

            # small weights first: phase0a only depends on these
            sGsWg = [wtile(t_GsWg[kt][:(128 if kt < 3 else 32)],
                           [(128 if kt < 3 else 32), 108], tag=f"GsWg{kt}")
                     for kt in range(4)]
            sGq = [wtile(t_Gq[kt][:(128 if kt < 3 else 32)],
                         [(128 if kt < 3 else 32), 64], tag=f"Gq{kt}")
                   for kt in range(4)]
            sSel = wtile(t_sel16, [16, 4], tag="sel16")
            sGb0 = wtile(t_gb0, [16, 1], F32, tag="gbias0")
            sGb1 = wtile(t_gb1, [12, 1], F32, tag="gbias1")
            sWmix = wtile(t_wmix, [128, 48], F32, tag="wmix")
            sId = wtile(t_ident, [128, 128], tag="ident")
            sS12 = wtile(t_s12, [12, 36], tag="s12")

            def load_wl0(kt, psplit=2):
                t = wpool.tile([128, 3072], BF16, tag=f"Wl0_{kt}",
                               name=f"Wl0_{kt}")
                pstep = 128 // psplit
                for ps in range(psplit):
                    pr = slice(ps * pstep, (ps + 1) * pstep)
                    nc.sync.dma_start(t[pr, :], t_Wl0[kt][pr, :])
                return t

            def load_late_weights():
                tb1 = wpool.tile([128, NE * 2 * 384], BF16, tag="Wb1P", name="Wb1P")
                nc.sync.dma_start(tb1[:], t_Wb1)
                sWb1 = [[tb1[:, (n * 2 + kt) * 384:(n * 2 + kt + 1) * 384]
                         for kt in range(2)] for n in range(NE)]
                t10 = wpool.tile([128, NE * 768], BF16, tag="W10P", name="W10P")
                nc.sync.dma_start(t10[:], t_W10)
                sW10 = [t10[:, n * 768:(n + 1) * 768] for n in range(NE)]
                t11 = wpool.tile([128, NE * 2 * 384], BF16, tag="W11P", name="W11P")
                nc.sync.dma_start(t11[:], t_W11)
                sW11 = [[t11[:, (n * 2 + kt) * 384:(n * 2 + kt + 1) * 384]
                         for kt in range(2)] for n in range(NE)]
                return sWb1, sW10, sW11

            # per-chunk state carried between phases
            xkq = [None] * NCHUNK     # [128, 1536] sparse ktiles 0..2
            hyb = [None] * NCHUNK     # [128, 512] hybrid ktile
            e0bf = [None] * NCHUNK    # [16, 512] mix-gate exps
            e1bf = [None] * NCHUNK    # [12, 512] domain-gate exps
            w016s = [None] * NCHUNK   # [16, 512] normalized mix gates
            w0bc = [None] * NCHUNK    # [128, 16*512] broadcast mix gates
            oht = [None] * NCHUNK     # 4x [128, 3] domain one-hot (batch-major)
            hA = [None] * NCHUNK      # 4x [128, 1024] L0b0 outputs (hh-paired)
            hB = [None] * NCHUNK      # 4x [128, 512] L0b1 outputs (pair slices)
            hBp = [None] * NCHUNK     # 2x [128, 1024] L0b1 npair tiles
            h2p_ = [None] * NCHUNK    # 2x [128, 1024] L1b1 npair tiles
            mixed = [None] * NCHUNK   # 4x [128, 512]
            hC = [None] * NCHUNK      # 4x [128, 1024]
            h2 = [None] * NCHUNK      # 4x [128, 512]

            # ===== P0a: loads, squares, fm, gate exps =====
            def phase0a_dma(ch, psplit=1):
                xk = xpool.tile([128, 3 * NBC], BF16, tag="xkq", name=f"xkq_{ch}")
                pstep = 128 // psplit
                for ps in range(psplit):
                    pr = slice(ps * pstep, (ps + 1) * pstep)
                    nc.sync.dma_start(xk[pr, :], t_xTk[ch][pr, :])
                xkq[ch] = xk
                hy = xpool.tile([128, NBC], BF16, tag="hyb", name=f"hyb_{ch}")
                nc.sync.dma_start(hy[0:32, :], t_xTs[ch])
                nc.vector.memset(hy[32:64, :], 0.0)
                nc.sync.dma_start(hy[32:45, :], t_dxT[ch])
                hyb[ch] = hy
                t = xpool.tile([12, NBC], BF16, tag="oh12", name=f"oh12_{ch}")
                nc.sync.dma_start(t[:], t_oh12[ch])
                oht[ch] = t

            def phase0a(ch):
                xk = xkq[ch]
                hy = hyb[ch]

                xq = qpool.tile([128, 3 * NBC], BF16, tag="xq", name=f"xq_{ch}")
                nc.scalar.activation(xq[:], xk[:], AF.Square)
                xqh = qpool.tile([32, NBC], BF16, tag="xqh", name=f"xqh_{ch}")
                nc.scalar.activation(xqh[:], hy[0:32, :], AF.Square)

                # merged s+gates matmul -> [92,512]; squares -> separate [64,512]
                gsw_ps = ps_smlt.tile([108, NBC], F32, tag="smlt", name=f"gsw_{ch}")
                for kt in range(3):
                    nc.tensor.matmul(gsw_ps[:], sGsWg[kt][:],
                                     xk[:, kt * NBC:(kt + 1) * NBC],
                                     start=(kt == 0), stop=False)
                nc.tensor.matmul(gsw_ps[:], sGsWg[3][:], hy[0:32, :],
                                 start=False, stop=True)
                gq_ps = ps_smlt.tile([64, NBC], F32, tag="smlt", name=f"gq_{ch}")
                for kt in range(3):
                    nc.tensor.matmul(gq_ps[:], sGq[kt][:],
                                     xq[:, kt * NBC:(kt + 1) * NBC],
                                     start=(kt == 0), stop=False)
                nc.tensor.matmul(gq_ps[:], sGq[3][:], xqh[:], start=False, stop=True)

                ssq = spool.tile([64, NBC], F32, tag="ssq", name=f"ssq_{ch}")
                nc.scalar.activation(ssq[:], gsw_ps[0:64, :], AF.Square,
                                     scale=float(np.sqrt(0.5)))
                nc.vector.tensor_tensor(hy[64:128, :], ssq[:], gq_ps[:], ALU.subtract)

                e0 = spool.tile([16, NBC], BF16, tag="e0bf", name=f"e0_{ch}")
                nc.scalar.activation(e0[:], gsw_ps[64:80, :], AF.Exp, bias=sGb0[:, 0:1])
                e0bf[ch] = e0
                e1 = gpool.tile([12, NBC], BF16, tag="e1bf", name=f"e1_{ch}")
                nc.scalar.activation(e1[:], gsw_ps[96:108, :], AF.Exp, bias=sGb1[:, 0:1])
                e1bf[ch] = e1

            # ===== P0b: softmax recip + gate-row broadcasts for mixing =====
            def phase0b(ch):
                e0 = e0bf[ch]
                s_ps = ps_smlt.tile([4, NBC], F32, tag="smlt", name=f"s0_{ch}")
                nc.tensor.matmul(s_ps[:], sSel[:], e0[:], start=True, stop=True)
                r0 = spool.tile([4, NBC], BF16, tag="r0", name=f"r0_{ch}")
                with nc.allow_low_precision("softmax recip feeds bf16 mix"):
                    nc.vector.reciprocal(r0[:], s_ps[:])
                r0rep = spool.tile([16, NBC], BF16, tag="r0rep", name=f"r0rep_{ch}")
                nc.sync.dma_start(r0rep[:], r0[:, None, :].broadcast_to([4, 4, NBC]))
                w016 = spool.tile([16, NBC], BF16, tag="w016", name=f"w016_{ch}")
                nc.vector.tensor_tensor(w016[:], e0[:], r0rep[:], ALU.mult)
                w016s[ch] = w016
                # stage the 16 gate rows on partition 0, then broadcast on GP
                wrow = bigpool.tile([1, 16 * NBC], BF16, tag="wrow", name=f"wrow_{ch}")
                nc.sync.dma_start(wrow[:], w016[:])
                wbig = bigpool.tile([128, 16 * NBC], BF16, tag="w0bc",
                                   name=f"w0bc_{ch}")
                for i in range(16):
                    nc.gpsimd.partition_broadcast(
                        wbig[:, i * NBC:(i + 1) * NBC],
                        wrow[0:1, i * NBC:(i + 1) * NBC])
                w0bc[ch] = wbig

            # -- one [128,1024] candidate pair eviction + accumulate into acc --
            def evict_c(ch, p, n, c, lb, acc, relu_act):
                wc = sWmix[:, lb + n * 3 + c: lb + n * 3 + c + 1]
                if c == 0:
                    if relu_act:
                        nc.scalar.activation(acc[:], p[:], AF.Relu, scale=wc)
                    else:
                        nc.vector.tensor_scalar(acc[:], p[:], 0.0, wc,
                                                ALU.max, ALU.mult)
                else:
                    fn = AF.Gelu_apprx_tanh if c == 1 else AF.Tanh
                    tg = "gt" if c == 1 else "tt"
                    t = apool.tile([128, 1024], BF16, tag=tg, name=f"{tg}{n}_{ch}")
                    nc.scalar.activation(t[:], p[:], fn)
                    t2 = apool.tile([128, 1024], BF16, tag="w" + tg,
                                    name=f"w{tg}{n}_{ch}")
                    nc.vector.tensor_scalar(t2[:], t[:], wc, None, ALU.mult)
                    nc.vector.tensor_tensor(acc[:], acc[:], t2[:], ALU.add)

            # -- one n-paired [128,1024] eviction for b1 layers --
            def evict_npair_c(ch, p, nlo, c, lb, outs):
                if c == 0:
                    rb = apool.tile([128, 1024], BF16, tag="rt", name=f"rt{nlo}_{ch}")
                    nc.scalar.activation(rb[:], p[:], AF.Relu)
                    tsrc = rb
                else:
                    fn = AF.Gelu_apprx_tanh if c == 1 else AF.Tanh
                    tg = "gt" if c == 1 else "tt"
                    t = apool.tile([128, 1024], BF16, tag=tg, name=f"{tg}b{nlo}_{ch}")
                    nc.scalar.activation(t[:], p[:], fn)
                    tsrc = t
                for half in range(2):
                    n = nlo + half
                    sl = slice(half * NBC, (half + 1) * NBC)
                    wc = sWmix[:, lb + n * 3 + c: lb + n * 3 + c + 1]
                    o = outs[n]
                    if c == 0:
                        nc.vector.tensor_scalar(o[:], tsrc[:, sl], wc, None, ALU.mult)
                    else:
                        nc.vector.scalar_tensor_tensor(o[:], tsrc[:, sl], wc, o[:],
                                                       ALU.mult, ALU.add)

            # ============ P1: L0b0 -> hA ; L0b1 -> hB ============
            def phase1_n(ch, n):
                if hA[ch] is None:
                    hA[ch] = {}
                    hB[ch] = {}
                    hBp[ch] = {}
                hA[ch][n] = hpool.tile([128, 1024], BF16, tag=f"hAC{n}",
                                       name=f"hA{n}_{ch}")
                for c in range(NC):
                    p = ps_mm.tile([128, 1024], F32, tag="pmm",
                                   name=f"pA{n}{c}_{ch}")
                    for hh in range(2):
                        m = n * 6 + c * 2 + hh
                        sl = slice(hh * NBC, (hh + 1) * NBC)
                        for kt in range(3):
                            nc.tensor.matmul(
                                p[:, sl], sWl0[kt][:, m * 128:(m + 1) * 128],
                                xkq[ch][:, kt * NBC:(kt + 1) * NBC],
                                start=(kt == 0), stop=False)
                        nc.tensor.matmul(
                            p[:, sl], sWl0[3][:, m * 128:(m + 1) * 128],
                            hyb[ch][:], start=False, stop=True)
                    evict_c(ch, p, n, c, 0, hA[ch][n], relu_act=True)

            def phase1_b1(ch, nlo):
                hbp = hpool.tile([128, 1024], BF16, tag=f"hBD{nlo}",
                                 name=f"hBp{nlo}_{ch}")
                hBp[ch][nlo // 2] = hbp
                for half in range(2):
                    hB[ch][nlo + half] = hbp[:, half * NBC:(half + 1) * NBC]
                for c in range(NC):
                    p = ps_mm.tile([128, 1024], F32, tag="pmm",
                                   name=f"pB{nlo}{c}_{ch}")
                    for half in range(2):
                        n = nlo + half
                        sl = slice(half * NBC, (half + 1) * NBC)
                        for kt in range(2):
                            nc.tensor.matmul(
                                p[:, sl], sWb1[n][kt][:, c * 128:(c + 1) * 128],
                                hA[ch][n][:, kt * NBC:(kt + 1) * NBC],
                                start=(kt == 0), stop=(kt == 1))
                    evict_npair_c(ch, p, nlo, c, 12, hB[ch])

            # ============ P2: expert mixing 0 (PE row-broadcast pairs) ============
            def phase2_n(ch, n):
                if mixed[ch] is None:
                    mixed[ch] = {}
                wb = w0bc[ch]
                i = n * 4
                p1 = bcpool.tile([128, 1024], BF16, tag="p2a",
                                 name=f"p1{n}_{ch}")
                nc.vector.tensor_tensor(p1[:], hBp[ch][0][:],
                                        wb[:, i * NBC:(i + 2) * NBC], ALU.mult)
                p2 = bcpool.tile([128, 1024], BF16, tag="p2b",
                                 name=f"p2{n}_{ch}")
                nc.vector.tensor_tensor(p2[:], hBp[ch][1][:],
                                        wb[:, (i + 2) * NBC:(i + 4) * NBC], ALU.mult)
                s = bcpool.tile([128, 1024], BF16, tag="p2s",
                                name=f"s{n}_{ch}")
                nc.vector.tensor_tensor(s[:], p1[:], p2[:], ALU.add)
                acc = bcpool.tile([128, NBC], BF16, tag=f"macc{n}",
                                  name=f"acc{n}_{ch}")
                nc.vector.tensor_tensor(acc[:], s[:, 0:NBC], s[:, NBC:2 * NBC],
                                        ALU.add)
                mixed[ch][n] = acc

            # ============ P3: L1b0 -> hC ; L1b1 -> h2 ============
            def phase3_n(ch, n):
                if hC[ch] is None:
                    hC[ch] = {}
                    h2[ch] = {}
                hC[ch][n] = hpool.tile([128, 1024], BF16, tag=f"hAC{n}",
                                       name=f"hC{n}_{ch}")
                for c in range(NC):
                    p = ps_mm.tile([128, 1024], F32, tag="pmm",
                                   name=f"pC{n}{c}_{ch}")
                    for hh in range(2):
                        mt = c * 2 + hh
                        sl = slice(hh * NBC, (hh + 1) * NBC)
                        nc.tensor.matmul(
                            p[:, sl], sW10[n][:, mt * 128:(mt + 1) * 128],
                            mixed[ch][n][:], start=True, stop=True)
                    evict_c(ch, p, n, c, 24, hC[ch][n], relu_act=True)

            def phase3_b1(ch, nlo):
                h2p = hpool.tile([128, 1024], BF16, tag=f"hBD{nlo}",
                                 name=f"h2p{nlo}_{ch}")
                if h2p_[ch] is None:
                    h2p_[ch] = {}
                h2p_[ch][nlo // 2] = h2p
                for half in range(2):
                    h2[ch][nlo + half] = h2p[:, half * NBC:(half + 1) * NBC]
                for c in range(NC):
                    p = ps_mm.tile([128, 1024], F32, tag="pmm",
                                   name=f"pD{nlo}{c}_{ch}")
                    for half in range(2):
                        n = nlo + half
                        sl = slice(half * NBC, (half + 1) * NBC)
                        for kt in range(2):
                            nc.tensor.matmul(
                                p[:, sl], sW11[n][kt][:, c * 128:(c + 1) * 128],
                                hC[ch][n][:, kt * NBC:(kt + 1) * NBC],
                                start=(kt == 0), stop=(kt == 1))
                    evict_npair_c(ch, p, nlo, c, 36, h2[ch])

            # ============ P4: domain softmax-select (batch-major) + out ============
            def phase4(ch):
                ws12 = bcpool.tile([12, NBC], BF16, tag="ws12", name=f"ws12_{ch}")
                nc.vector.tensor_tensor(ws12[:], oht[ch][:], e1bf[ch][:], ALU.mult)
                s36 = ps_smlt.tile([36, NBC], F32, tag="smlt", name=f"s36_{ch}")
                nc.tensor.matmul(s36[:], sS12[:], ws12[:], start=True, stop=True)
                rn = opool.tile([1, NBC], F32, tag="rn", name=f"rn_{ch}")
                nc.vector.reciprocal(rn[:], s36[0:1, :])
                es4 = opool.tile([4, NBC], BF16, tag="es4", name=f"es4_{ch}")
                nc.vector.tensor_copy(es4[:], s36[32:36, :])
                wsrow = bigpool.tile([1, 4 * NBC], BF16, tag="wsrow",
                                     name=f"wsrow_{ch}")
                nc.sync.dma_start(wsrow[:], es4[:])
                wsbig = bigpool.tile([128, 4 * NBC], BF16, tag="wsbig",
                                     name=f"wsbig_{ch}")
                for e in range(NE):
                    nc.gpsimd.partition_broadcast(
                        wsbig[:, e * NBC:(e + 1) * NBC],
                        wsrow[0:1, e * NBC:(e + 1) * NBC])
                rnbig = bcpool.tile([128, NBC], F32, tag="rnbig", name=f"rnb_{ch}")
                nc.gpsimd.partition_broadcast(rnbig[:], rn[:])
                p1 = bcpool.tile([128, 1024], BF16, tag="p2a", name=f"q1_{ch}")
                nc.vector.tensor_tensor(p1[:], h2p_[ch][0][:],
                                        wsbig[:, 0:1024], ALU.mult)
                p2 = bcpool.tile([128, 1024], BF16, tag="p2b", name=f"q2_{ch}")
                nc.vector.tensor_tensor(p2[:], h2p_[ch][1][:],
                                        wsbig[:, 1024:2048], ALU.mult)
                s = bcpool.tile([128, 1024], BF16, tag="p2s", name=f"qs_{ch}")
                nc.vector.tensor_tensor(s[:], p1[:], p2[:], ALU.add)
                accm = bcpool.tile([128, NBC], BF16, tag="oaccm", name=f"oam_{ch}")
                nc.vector.tensor_tensor(accm[:], s[:, 0:NBC], s[:, NBC:2 * NBC],
                                        ALU.add)
                fin = bcpool.tile([128, NBC], BF16, tag="ofin", name=f"ofin_{ch}")
                nc.vector.tensor_tensor(fin[:], accm[:], rnbig[:], ALU.mult)
                tp = ps_smlt.tile([128, 4 * OUT], BF16, tag="smlt", name=f"tp_{ch}")
                for bt in range(4):
                    nc.tensor.transpose(tp[:, bt * OUT:(bt + 1) * OUT],
                                        fin[:, bt * 128:(bt + 1) * 128], sId[:])
                ot = opool.tile([128, 4 * OUT], F32, tag="otile", name=f"ot_{ch}")
                nc.vector.tensor_copy(ot[:], tp[:])
                nc.sync.dma_start(t_out[ch], ot[:])

            # ---- emission schedule: P0a all, big weights, interleaved rounds ----
            phase0a_dma(0, psplit=4)
            phase0a_dma(1, psplit=2)
            sWl0 = [load_wl0(kt) for kt in range(4)]
            phase0a(0)
            phase0a_dma(2)
            phase0a(1)
            phase0a(2)
            sWb1, sW10, sW11 = load_late_weights()

            def round_(ch):
                # ch: current chunk for P1; ch-1 for P2/P3; ch-2 for P4
                if ch > 1:
                    phase4(ch - 2)
                phase0b(ch)
                phase1_n(ch, 0)
                if ch > 0:
                    phase2_n(ch - 1, 0)
                phase1_n(ch, 1)
                if ch > 0:
                    phase2_n(ch - 1, 1)
                phase1_b1(ch, 0)
                phase1_n(ch, 2)
                if ch > 0:
                    phase2_n(ch - 1, 2)
                phase1_n(ch, 3)
                if ch > 0:
                    phase2_n(ch - 1, 3)
                phase1_b1(ch, 2)
                if ch > 0:
                    phase3_n(ch - 1, 0)
                    phase3_n(ch - 1, 1)
                    phase3_b1(ch - 1, 0)
                    phase3_n(ch - 1, 2)
                    phase3_n(ch - 1, 3)
                    phase3_b1(ch - 1, 2)

            for ch in range(NCHUNK):
                if ch == 1:
                    phase0a_dma(3)
                    phase0a(3)
                round_(ch)
            # epilogue: finish chunk 2 and 3 tails, interleaved
            last = NCHUNK - 1
            phase4(last - 1)
            phase2_n(last, 0)
            phase2_n(last, 1)
            phase2_n(last, 2)
            phase2_n(last, 3)
            phase3_n(last, 0)
            phase3_n(last, 1)
            phase3_b1(last, 0)
            phase3_n(last, 2)
            phase3_n(last, 3)
            phase3_b1(last, 2)
            phase4(last)
    nc.compile()
    return nc


_CACHE = {}


def kernel(**inputs):
    assert (np.abs(inputs['b_l0b0']).max() == 0.0
            and np.abs(inputs['b_l0b1']).max() == 0.0
            and np.abs(inputs['b_l1b0']).max() == 0.0
            and np.abs(inputs['b_l1b1']).max() == 0.0), \
        "optimized kernel assumes zero expert biases"
    shared = prep_shared(inputs)
    in_maps = []
    for r in range(N_CORES):
        m = dict(shared)
        m.update(prep_core(inputs, r))
        in_maps.append(m)
    if 'nc' not in _CACHE:
        _CACHE['nc'] = build_program()
    nc = _CACHE['nc']
    res = run_bass_kernel_spmd(nc, in_maps, core_ids=list(range(N_CORES)))
    outs = []
    for r in range(N_CORES):
        o = res.results[r]['out'].reshape(NCHUNK, 128, 4, OUT)
        outs.append(o.transpose(0, 2, 1, 3).reshape(B_LOC, OUT))
    return np.concatenate(outs, axis=0).astype(np.float32)
